# revision 35
# baseline (speedup 1.0000x reference)
"""Bass/Trainium2 kernel for nn_BinResNetConv2d.

Computes: BatchNorm2d (inference) -> sign binarization -> 3x3 conv
(256->256 ch, stride 1, pad 1, no bias) -> ReLU on x[32, 256, 56, 56].

Strategy: data-parallel over batch across 8 NeuronCores (4 images/core,
conv weights + BN params replicated). Per core:
  - BN is folded on host into per-channel (scale, shift); ScalarE
    computes sign(x*scale + shift) per tile, writing fp8 +/-1 into a
    zero-padded flat-row image S1 [128ci, 2ci_t, 59x57]; a second
    ScalarE Copy produces S2 = S1 * 2^-9 (+/-2^-9, exact in fp8).
    Rows stride 57: each row is [zero | 56 data], the zero doubling as
    right pad of the previous row and left pad of its own.
  - The 3x3 conv runs on fp8 DoubleRow matmuls: one instruction
    contracts BOTH 128-channel ci tiles (2 K-slots) at the bf16
    single-slot rate => 2x tensor-engine throughput (157 TF/s).
    Per 8-row output tile: 9 "hi" taps vs S1 with e4m3(w), plus
    N_FIX "lo" taps vs S2 with e4m3((w - hi) * 2^9) which restore
    precision on the first N_FIX taps.  Moving operand is one flat
    contiguous 456-run (8 rows x 57) so only 3-dim APs are needed;
    the garbage column per row is never copied out.
  - ReLU on VectorE evacuates PSUM -> SBUF f32 (row-tiles paired into
    16-row stores).  A store dma_start BLOCKS its engine until the
    evac semaphore fires, so y-stores ride the sync ring (idle after
    the early image-0 loads) while the stage loads for images 1..3 go
    on the ScalarE ring; emission is interleaved by image so every
    queue drains in pipeline order.

Accuracy (measured against the jax reference on the real inputs):
N_FIX=5 -> rel err ~1.7e-2; N_FIX=6 -> ~1.5e-2; N_FIX=9 (full hi/lo)
-> ~6e-4; N_FIX=0 -> 2.5e-2 (fails the 2e-2 gate).  fp8 sign values
are exact, so the only error is e4m3 weight rounding on the 9-N_FIX
uncorrected taps.
"""

import numpy as np
import ml_dtypes

N_CORES = 8
NB = 4              # images per core (32 / 8)
C = 256
H = W = 56
# shared-pad flat layout: each 57-cell row is [zero | 56 data]; the zero
# is simultaneously the LEFT pad of its row and the RIGHT pad of the
# previous row, so rows stride 57 instead of 58 (-1.7% matmul time)
WP = 57
# 1 pre-pad row + 56 data + 1 post-pad + 2 slack rows.  60 (not 59):
# the DoubleRow matmul requires an EVEN byte stride between its two
# K-tile planes (59*57 = 3363 hard-faults the NEFF at runtime).
NROWS = 60
NTAP = 9
N_FIX = 5           # correction (lo) taps per tile; taps 0..N_FIX-1
NI = NTAP + N_FIX   # DoubleRow matmuls per output tile
LO_SCALE = 512.0    # lo weights are e4m3(resid * 512), S2 = +/-2^-9
ROWS_PER_TILE = 8
N_ROW_TILES = H // ROWS_PER_TILE  # 7
NMOV = ROWS_PER_TILE * WP         # 464 moving elements per matmul

_nc_cache = {}
LAST_RESULTS = None


def _build_nc():
    import concourse.mybir as mybir
    import concourse.tile as tile
    from concourse import bacc
    from concourse.tile import add_dep_helper

    f32 = mybir.dt.float32
    bf16 = mybir.dt.bfloat16
    fp8 = mybir.dt.float8e4
    AF = mybir.ActivationFunctionType
    DR = mybir.MatmulPerfMode.DoubleRow

    nc = bacc.Bacc("TRN2", target_bir_lowering=False, debug=False)
    x_d = nc.dram_tensor("x", (NB, C, H, W), f32, kind="ExternalInput")
    # wt[ci, co_t, it, ci_t, co]: lhsT slices; it<9 = hi tap it (vs S1),
    # it>=9 = lo tap it-9 (vs S2)
    wt_d = nc.dram_tensor("wt", (128, 2, NI, 2, 128), fp8,
                          kind="ExternalInput")
    bnp_d = nc.dram_tensor("bnp", (2, 128, 2), f32, kind="ExternalInput")
    y_d = nc.dram_tensor("y", (NB, C, H, W), f32, kind="ExternalOutput")

    with tile.TileContext(nc) as tc:
        with (
            tc.tile_pool(name="const", bufs=1) as cpool,
            tc.tile_pool(name="xp", bufs=1) as xpool,
            tc.tile_pool(name="chunk", bufs=8) as hpool,
            tc.tile_pool(name="stage", bufs=6) as spool,
            tc.tile_pool(name="out", bufs=6) as opool,
            tc.tile_pool(name="psum", bufs=8, space="PSUM") as ppool,
        ):
            # zero scratch for PE warm-up matmuls (HAM un-throttles after
            # ~3.4us of sustained PE work; run it on zeros while x loads).
            warm_sb = cpool.tile([128, 256], bf16, tag="warm")
            nc.gpsimd.memset(warm_sb[:], 0.0)
            # BN params: sole first transfer on the ScalarE HW-DGE ring so
            # nothing can starve it (the first Sign waits on it)
            bnp_sb = []  # [128, 2]: col 0 = scale, col 1 = shift
            for ci_t in range(2):
                t = cpool.tile([128, 2], f32, tag=f"bnp{ci_t}")
                nc.scalar.dma_start(t[:], bnp_d[ci_t])
                bnp_sb.append(t)
            wt_sb = cpool.tile([128, 2, NI, 2, 128], fp8, tag="wt",
                               name="wt")

            # --- padded fp8 sign images; borders + slack row zeroed
            # (disjoint from the interior Sign writes, so no dep lands on
            # the Activation ops). S2 = S1 * 2^-9 feeds the lo taps.
            s_img = {}   # (n, lvl) -> [128, 2, 59, 57] fp8
            for n in range(NB):
                for lvl in range(2):
                    t = xpool.tile([128, 2, NROWS, WP], fp8,
                                   tag=f"s{lvl}_{n}")
                    nc.gpsimd.memset(t[:, :, 0, :], 0.0)
                    nc.gpsimd.memset(t[:, :, 57:NROWS, :], 0.0)
                    nc.gpsimd.memset(t[:, :, 1:57, 0], 0.0)
                    s_img[(n, lvl)] = t
            # flat views for the matmul moving operands
    # (merged (row, col) -> one contiguous dim)
            s_flat = {k: v[:].rearrange("p a r c -> p a (r c)")
                      for k, v in s_img.items()}

            def sign_rows(n, ci_t, r0, nr, src_ap):
                """rows [r0, r0+nr): S1 = sign(x*scale+shift), fp8 +/-1."""
                dst1 = s_img[(n, 0)][:, ci_t, 1 + r0:1 + r0 + nr, 1:WP]
                nc.scalar.activation(
                    dst1, src_ap, AF.Sign,
                    bias=bnp_sb[ci_t][:, 1:2], scale=bnp_sb[ci_t][:, 0:1])

            def copy_rows(n, ci_t, r0, nr):
                """S2 = S1 * 2^-9 (+/-2^-9, exact in fp8) for those rows."""
                src = s_img[(n, 0)][:, ci_t, 1 + r0:1 + r0 + nr, 1:WP]
                dst2 = s_img[(n, 1)][:, ci_t, 1 + r0:1 + r0 + nr, 1:WP]
                nc.scalar.activation(dst2, src, AF.Copy, scale=2.0 ** -9)

            # image 0 in row-chunks per ci tile: chunk 0 (10 rows) covers
            # every matmul of the first output tile; each following 8-row
            # chunk unblocks exactly one more row-tile, so the PE (2.9us
            # per tile) never outruns the DMA (~1.4us per chunk).
            # HBM is fair-shared across active DMA queues, so launching
            # everything at once makes the conv-critical first transfers
            # ~5x slower. Issue the SP ring in waves: each wave's first
            # transfer must complete before the next wave may issue.
            CHUNK_ROWS = [(10, 0), (8, 10), (8, 18), (8, 26), (8, 34),
                          (8, 42), (6, 50)]

            def chunk_dma(c, ci_t, nsub=1, eng=None):
                """Load + binarize chunk c of image 0.  The S2 copies for
                both ci tiles are emitted after ci_t=1's Sign so the first
                DoubleRow matmul (which pairs BOTH ci tiles) never waits
                behind a Copy on the serial ScalarE queue."""
                eng = eng or nc.sync
                nr, r = CHUNK_ROWS[c]
                h = nr // 2 if nsub == 2 else nr
                st = hpool.tile([128, 14, W], f32, tag="chunk", name="st")
                dma = eng.dma_start(
                    st[:, 0:h, :],
                    x_d[0, ci_t * 128:(ci_t + 1) * 128, r:r + h, :])
                if nsub == 2:
                    eng.dma_start(
                        st[:, h:nr, :],
                        x_d[0, ci_t * 128:(ci_t + 1) * 128, r + h:r + nr, :])
                sign_rows(0, ci_t, r, nr, st[:, 0:nr, :])
                if ci_t == 1:
                    copy_rows(0, 0, r, nr)
                    copy_rows(0, 1, r, nr)
                return dma

            def stage_pair(n):
                """Image n (1..3) in 14-row strips, ci tiles interleaved:
                the first DoubleRow matmul of image n needs S1+S2 of
                strip 0 for BOTH ci tiles, which are the first four
                ScalarE ops of this block -- the rest pipelines behind."""
                sts = []
                for ci_t in range(2):
                    st = spool.tile([128, H, W], f32, tag="stage",
                                    name="st")
                    sts.append(st)
                for r in range(0, H, 14):
                    for ci_t in range(2):
                        nc.scalar.dma_start(
                            sts[ci_t][:, r:r + 14, :],
                            x_d[n, ci_t * 128:(ci_t + 1) * 128,
                                r:r + 14, :])
                    for ci_t in range(2):
                        sign_rows(n, ci_t, r, 14, sts[ci_t][:, r:r + 14, :])
                        copy_rows(n, ci_t, r, 14)

            def wt_dma(co_t, its=(0, NI), eng=None):
                """Weight load for instruction range [lo, hi) of co_t."""
                eng = eng or nc.sync
                lo, hi = its
                return eng.dma_start(
                    wt_sb[:, co_t, lo:hi], wt_d[:, co_t, lo:hi])

            # Image-0 loads go out first, in two waves: wave 2's first
            # transfer must wait for wave 1's first chunk so the
            # conv-critical first rows keep HBM priority.
            # chunk0's two ci halves go out on DIFFERENT rings (scalar +
            # sync) so their queue inits and transfers run in parallel.
            # The ky=0 weights ride the scalar ring ahead of chunk0-ci0,
            # ungated: they're tiny, fire at boot, and only consume the
            # slack of the EARLIER sign (ci0) -- mm#0 is gated by ci1.
            wt_dma(0, its=(0, 3), eng=nc.scalar)
            wave1 = [lambda: chunk_dma(0, 0, nsub=2, eng=nc.scalar),
                     lambda: chunk_dma(0, 1, nsub=2)]
            wave2 = [lambda: wt_dma(0, its=(3, NI)),
                     lambda: chunk_dma(1, 0), lambda: chunk_dma(1, 1),
                     lambda: chunk_dma(2, 0), lambda: chunk_dma(2, 1),
                     lambda: wt_dma(1, its=(0, 6)),
                     lambda: chunk_dma(3, 0), lambda: chunk_dma(3, 1),
                     lambda: chunk_dma(4, 0), lambda: chunk_dma(4, 1),
                     lambda: wt_dma(1, its=(6, NI)),
                     lambda: chunk_dma(5, 0), lambda: chunk_dma(5, 1),
                     lambda: chunk_dma(6, 0), lambda: chunk_dma(6, 1)]
            gate = None
            for emit in wave1:
                dma = emit()
                if gate is None:
                    gate = dma
            for emit in wave2:
                dma = emit()
                add_dep_helper(dma.ins, gate.ins, sync=True,
                               reason="DMA wave schedule")

            # PE warm-up: zero matmuls keep the PE's activity monitor busy
            # from ~7us until the first real matmul, so conv starts at the
            # full 2.4GHz clock instead of the 1.2GHz cold state
            warm_ps = ppool.tile([128, ROWS_PER_TILE, WP], f32, tag="ps")
            warm_pm = warm_ps[:].rearrange("p a b -> p (a b)")
            last_warm = None
            for _ in range(17):
                last_warm = nc.tensor.matmul(
                    warm_pm[0:64, 0:256], warm_sb[:, 0:64], warm_sb[:])

            # --- conv: NI DoubleRow matmuls per output tile ---
            # Emission is interleaved BY IMAGE: image n's tiles (evac +
            # y-store instructions) are emitted before image n+1's stage
            # loads + signs.  Engine instruction queues and DGE rings both
            # drain in program order, so emitting all loads first would
            # park every y-store enqueue behind ~40us of stage Sign work
            # on ScalarE (and the stage transfers ahead of stores on the
            # ring), starving the out-buffer pool and stalling the PE on
            # PSUM recycling.
            n_tiles = NB * 2 * N_ROW_TILES
            ti = 0
            first_mm = None
            pend = [None]   # 16-row out tile awaiting its second half

            def conv_image(n):
                nonlocal ti, first_mm
                for co_t in range(2):
                    co_sl = slice(co_t * 128, (co_t + 1) * 128)
                    for rb in range(N_ROW_TILES):
                        r0 = rb * ROWS_PER_TILE
                        ps = ppool.tile([128, ROWS_PER_TILE, WP], f32,
                                        tag="ps")
                        pm = ps[:].rearrange("p a b -> p (a b)")
                        # 9 hi taps first, lo fixes last -> each tile has
                        # ~1.8us of hi runway before needing the S2 copies
                        for k in range(NI):
                            tap = k if k < NTAP else k - NTAP
                            lvl = 0 if k < NTAP else 1
                            ky, kx = divmod(tap, 3)
                            off = (r0 + ky) * WP + kx
                            rhs = s_flat[(n, lvl)][:, :, off:off + NMOV]
                            mm = nc.tensor.matmul(
                                pm, wt_sb[:, co_t, k], rhs,
                                start=(k == 0), stop=(k == NI - 1),
                                perf_mode=DR)
                            if first_mm is None:
                                first_mm = mm
                        ti += 1
                        if ti == n_tiles:
                            # final tiles: evacuate + store in halves across
                            # both DMA rings so the kernel tail pipelines
                            ob = opool.tile([128, ROWS_PER_TILE, W], f32,
                                            tag="obf")
                            half = ROWS_PER_TILE // 2
                            nc.vector.tensor_scalar_max(
                                ob[:, 0:half, :], ps[:, 0:half, 0:W], 0.0)
                            nc.sync.dma_start(
                                y_d[n, co_sl, r0:r0 + half, :],
                                ob[:, 0:half, :])
                            nc.vector.tensor_scalar_max(
                                ob[:, half:ROWS_PER_TILE, :],
                                ps[:, half:ROWS_PER_TILE, 0:W], 0.0)
                            nc.scalar.dma_start(
                                y_d[n, co_sl, r0 + half:r0 + ROWS_PER_TILE, :],
                                ob[:, half:ROWS_PER_TILE, :])
                        elif rb % 2 == 0 and rb < N_ROW_TILES - 1:
                            # even rb: evacuate into the first half of a
                            # 16-row out tile; the paired store goes out
                            # with the next row-tile (halves the number of
                            # y-store enqueues on the ScalarE queue)
                            pend[0] = opool.tile(
                                [128, 2 * ROWS_PER_TILE, W], f32, tag="ob",
                                name="ob2")
                            nc.vector.tensor_scalar_max(
                                pend[0][:, 0:ROWS_PER_TILE, :],
                                ps[:, :, 0:W], 0.0)
                        else:
                            if rb % 2 == 1:
                                ob2 = pend[0]
                                nc.vector.tensor_scalar_max(
                                    ob2[:, ROWS_PER_TILE:, :],
                                    ps[:, :, 0:W], 0.0)
                                # y-stores ride the sync HW-DGE ring: a
                                # store dma_start BLOCKS its engine until
                                # the evac semaphore fires, so they must
                                # not share an engine with the Sign work;
                                # stage loads go on the scalar ring instead
                                nc.sync.dma_start(
                                    y_d[n, co_sl,
                                        r0 - ROWS_PER_TILE:r0 + ROWS_PER_TILE,
                                        :],
                                    ob2[:, :, :])
                            else:
                                # rb == 6 singleton
                                ob = opool.tile([128, ROWS_PER_TILE, W],
                                                f32, tag="obf")
                                nc.vector.tensor_scalar_max(
                                    ob[:], ps[:, :, 0:W], 0.0)
                                nc.sync.dma_start(
                                    y_d[n, co_sl, r0:r0 + ROWS_PER_TILE, :],
                                    ob[:])

            conv_image(0)
            for n in range(1, NB):
                stage_pair(n)
                conv_image(n)
            # keep warm-up strictly before the real matmuls on the PE queue
            add_dep_helper(first_mm.ins, last_warm.ins, sync=False,
                           reason="PE warm-up precedes conv")
    nc.compile()
    return nc


def _get_nc():
    if "nc" not in _nc_cache:
        _nc_cache["nc"] = _build_nc()
    return _nc_cache["nc"]


def kernel(x, w, gamma, beta, running_mean, running_var, _trace=False):
    global LAST_RESULTS
    from concourse.bass_utils import run_bass_kernel_spmd

    x = np.ascontiguousarray(np.asarray(x, dtype=np.float32))
    w = np.asarray(w, dtype=np.float32)
    gamma = np.asarray(gamma, dtype=np.float32)
    beta = np.asarray(beta, dtype=np.float32)
    running_mean = np.asarray(running_mean, dtype=np.float32)
    running_var = np.asarray(running_var, dtype=np.float32)

    # fold BN (inference) into per-channel scale/shift
    eps = 1e-5
    scale = gamma / np.sqrt(running_var + eps)
    shift = beta - running_mean * scale

    # weights -> hi = e4m3(w), lo = e4m3((w - hi) * 512) for taps
    # 0..N_FIX-1; lhsT layout [ci, co_t, it, ci_t, co]
    fp8 = ml_dtypes.float8_e4m3
    hi8 = w.astype(fp8)
    resid = w - hi8.astype(np.float32)
    lo8 = (resid * LO_SCALE).astype(fp8)

    def to_lhsT(q):   # [co, ci, ky, kx] (fp8) -> [ci, co_t, tap, ci_t, co]
        q6 = np.asarray(q).reshape(2, 128, 2, 128, 3, 3)
        return q6.transpose(3, 0, 4, 5, 2, 1).reshape(128, 2, 9, 2, 128)

    hi_t = to_lhsT(hi8)
    lo_t = to_lhsT(lo8)[:, :, :N_FIX]
    wt = np.ascontiguousarray(
        np.concatenate([hi_t, lo_t], axis=2))   # [128, 2, NI, 2, 128]

    nc = _get_nc()
    bnp = np.ascontiguousarray(
        np.stack([scale, shift], axis=-1).reshape(2, 128, 2).astype(np.float32))
    in_maps = [
        {
            "x": np.ascontiguousarray(x[i * NB:(i + 1) * NB]),
            "wt": wt,
            "bnp": bnp,
        }
        for i in range(N_CORES)
    ]
    res = run_bass_kernel_spmd(nc, in_maps, core_ids=list(range(N_CORES)),
                               trace=_trace)
    LAST_RESULTS = res
    y = np.concatenate([r["y"] for r in res.results], axis=0)
    return y
